# revision 33
# baseline (speedup 1.0000x reference)
"""Trainium2 Bass kernel for the histogram_binning problem.

Input  x: [16, 512, 128, 128] f32.  Output: [16, 4096] f32.

Per batch b:
  vmax[c]  = max over (H,W) of x[b,c]
  rc[h]    = #channels whose argmax row is h (matches the reference's
             argmax-indicator row counts for tie-free-within-row data)
  Hc       = exclusive cumsum of rc
  bin edges hk[k] via the reference's sequential threshold scan, computed in
  closed form: J_k = #{t in 0..126 : HcInc[t] <= 64k}; a bin "fires" iff all
  previous fired, J strictly increases, and J_k <= 126.
  F[c]     = sum over (H,W) of x[b,c] / W
  out[b, k*512 + c] = F[c] / (hk[k+1] - hk[k])

Sharding: pure data parallel, 2 batches per core across 8 cores.

Per-core device pipeline (x shard [2, 512, 128, 128], streamed as 8 channel
tiles x 4 H-slices of [128c, 32h, 128w], 8-deep buffered, one gapless HWDGE
DMA stream on the sync ring):
  DMA in -> DVE row-max into M1 slices (concurrent) ACT copy-to-scratch with
  accum (per-channel sum S) -> DVE vmax -> DVE rowhit = (M1 >= vmax) -> PE
  matmul ones^T @ rowhit accumulating rc in PSUM.
Epilogues (both batches, deferred past the streaming loop so their latency
chain never starves the DMA queue): DVE scan for the cumulative histogram,
PE broadcast + one fused DVE compare-count for the 7 bin edges, fired-prefix
via mult-scan, reciprocal, PE broadcast of 1/hk_sub, PE transpose of the
[128c, 32] result block, one contiguous DMA out per batch.
"""

import numpy as np

B, C, H, W = 16, 512, 128, 128
NS = 8
NCORES = 8
PB = B // NCORES  # batches per core

_NC = None


def _build(xbufs=8, nh=4, inline_b0=False, scr_bf16=False):
    from concourse import bacc, tile, mybir
    from concourse.bass_primitives import MemorySpace

    f32 = mybir.dt.float32
    Alu = mybir.AluOpType
    Act = mybir.ActivationFunctionType
    Ax = mybir.AxisListType

    nc = bacc.Bacc(None, target_bir_lowering=False)

    x_ext = nc.dram_tensor("x", [PB, C, H, W], f32, kind="ExternalInput")
    cones_ext = nc.dram_tensor("cones", [128, 1], f32, kind="ExternalInput")
    rones_ext = nc.dram_tensor("rones", [1, 128], f32, kind="ExternalInput")
    ident_ext = nc.dram_tensor("ident", [128, 128], f32, kind="ExternalInput")
    thr8_ext = nc.dram_tensor("thr8", [8, 1], f32, kind="ExternalInput")
    out_ext = nc.dram_tensor("out", [PB, NS * C], f32, kind="ExternalOutput")
    # out flat index = k*512 + q*128 + c with q = channel-tile index
    out_v = out_ext.rearrange("b (k q c) -> b q k c", k=NS, q=4, c=128)

    NT = C // 128  # channel tiles per batch

    with tile.TileContext(nc) as tc:
        with (
            tc.tile_pool(name="consts", bufs=1) as cp,
            tc.tile_pool(name="xp", bufs=xbufs) as xp,
            tc.tile_pool(name="sp", bufs=3) as sp,
            tc.tile_pool(name="ep", bufs=2) as ep,
            tc.tile_pool(name="pp", bufs=2, space=MemorySpace.PSUM) as pp,
        ):
            cones = cp.tile([128, 1], f32)
            nc.gpsimd.dma_start(out=cones[:], in_=cones_ext[:])
            rones = cp.tile([1, 128], f32)
            nc.gpsimd.dma_start(out=rones[:], in_=rones_ext[:])
            ident = cp.tile([128, 128], f32)
            nc.gpsimd.dma_start(out=ident[:], in_=ident_ext[:])
            thr8 = cp.tile([8, 1], f32)
            nc.gpsimd.dma_start(out=thr8[:], in_=thr8_ext[:])
            # ACT sum scratch: the Copy-with-accum needs a full-size out
            # stream; write it to a reusable scratch so X stays read-only
            # (keeps DVE and ACT concurrent on the same tile).
            NH = nh  # H-slices per channel tile (DMA granularity)
            HH = H // NH
            scr_dt = mybir.dt.bfloat16 if scr_bf16 else f32
            SCR_E = min(HH * W, 8192)
            scr_act = cp.tile([128, SCR_E], scr_dt)

            # ---- per-batch epilogues (deferred off the streaming path) ----
            def epilogue(b):
                S_b = S_bs[b]
                psum_rc = psum_rcs[b]
                HcInc = ep.tile([1, H], f32, tag="HcInc")
                nc.vector.tensor_tensor_scan(
                    out=HcInc[:], data0=psum_rc[:], data1=rones[:],
                    initial=0.0, op0=Alu.add, op1=Alu.mult,
                )
                # J_k = #{t in 0..H-2 : HcInc[t] <= 64k}: broadcast HcInc to 8
                # partitions (PE), one fused compare+count (DVE), transpose
                # back to the free axis (PE).
                psum_hc8 = pp.tile([8, H], f32, tag="hc8", bufs=1)
                nc.tensor.matmul(psum_hc8[:], rones[0:1, 0:8], HcInc[:],
                                 start=True, stop=True)
                J_pp = ep.tile([8, 1], f32, tag="J_pp")
                scr8 = ep.tile([8, H - 1], f32, tag="scr8")
                nc.vector.tensor_scalar(
                    out=scr8[:], in0=psum_hc8[0:8, 0:H - 1], scalar1=thr8[:],
                    scalar2=None, op0=Alu.is_le, op1=Alu.add,
                    accum_out=J_pp[:],
                )
                psum_J = pp.tile([1, 8], f32, tag="psum_J", bufs=1)
                nc.tensor.matmul(psum_J[:], J_pp[:], ident[0:8, 0:8],
                                 start=True, stop=True)
                Jb = ep.tile([1, 8], f32, tag="Jb")
                nc.vector.memset(Jb[:], 0.0)
                nc.vector.tensor_copy(out=Jb[0:1, 1:8], in_=psum_J[0:1, 0:7])
                inc = ep.tile([1, 7], f32, tag="inc")
                nc.vector.tensor_tensor(out=inc[:], in0=Jb[0:1, 1:8], in1=Jb[0:1, 0:7], op=Alu.is_gt)
                le = ep.tile([1, 7], f32, tag="le")
                nc.vector.tensor_scalar(
                    out=le[:], in0=Jb[0:1, 1:8], scalar1=float(H - 2),
                    scalar2=None, op0=Alu.is_le,
                )
                inc2 = ep.tile([1, 7], f32, tag="inc2")
                nc.vector.tensor_tensor(out=inc2[:], in0=inc[:], in1=le[:], op=Alu.mult)
                fired = ep.tile([1, 7], f32, tag="fired")
                nc.vector.tensor_tensor_scan(
                    out=fired[:], data0=inc2[:], data1=rones[0:1, 0:7],
                    initial=1.0, op0=Alu.mult, op1=Alu.mult,
                )
                hks = ep.tile([1, NS + 1], f32, tag="hks")
                nc.vector.memset(hks[:], 0.0)
                nc.vector.memset(hks[0:1, NS:NS + 1], float(H))
                nc.vector.tensor_tensor(out=hks[0:1, 1:8], in0=Jb[0:1, 1:8], in1=fired[:], op=Alu.mult)
                hsub = ep.tile([1, NS], f32, tag="hsub")
                nc.vector.tensor_tensor(out=hsub[:], in0=hks[0:1, 1:9], in1=hks[0:1, 0:8], op=Alu.subtract)
                inv = ep.tile([1, NS], f32, tag="inv")
                nc.vector.reciprocal(out=inv[:], in_=hsub[:])

                # broadcast inv across 128 partitions: [1,128] ones^T @ [1,8]
                psum_inv = pp.tile([128, NS], f32, tag="pinv", bufs=2)
                nc.tensor.matmul(psum_inv[:], rones[:], inv[:], start=True, stop=True)

                outt = ep.tile([128, NT, NS], f32, tag="outt")
                for ct in range(NT):
                    nc.vector.tensor_scalar(
                        out=outt[:, ct, :], in0=psum_inv[:],
                        scalar1=S_b[:, ct:ct + 1], scalar2=1.0 / float(W),
                        op0=Alu.mult, op1=Alu.mult,
                    )
                # transpose [128c, (q,k)=32] -> [32, 128c] via PE with identity
                psum_o = pp.tile([NT * NS, 128], f32, tag="po", bufs=2)
                nc.tensor.matmul(psum_o[:], outt[:], ident[:], start=True, stop=True)
                outT = ep.tile([NT * NS, 128], f32, tag="outT")
                nc.vector.tensor_copy(out=outT[:], in_=psum_o[:])
                nc.sync.dma_start(out=out_v[b], in_=outT[:])


            S_bs, psum_rcs = [], []
            for b in range(PB):
                S_b = ep.tile([128, NT], f32, tag="S_b")
                psum_rc = pp.tile([1, H], f32, tag="rc", bufs=2)
                S_bs.append(S_b)
                psum_rcs.append(psum_rc)
                for ct in range(NT):
                    # finer slices on the last tile -> shorter pipeline trail
                    nh = 8 if (b == PB - 1 and ct == NT - 1) else NH
                    hs = H // nh
                    M1 = sp.tile([128, H], f32, tag="M1")
                    nchunks = max(1, (H * W // nh) // SCR_E)
                    S_parts = sp.tile([128, 8], f32, tag="S_parts")
                    if nh * nchunks < 8:
                        nc.vector.memset(S_parts[:, nh * nchunks:8], 0.0)
                    assert nh * nchunks <= 8
                    for hh in range(nh):
                        X = xp.tile([128, HH, W], f32, tag="X")
                        nc.sync.dma_start(
                            out=X[:, 0:hs, :],
                            in_=x_ext[b, ct * 128:(ct + 1) * 128,
                                      hh * hs:(hh + 1) * hs])
                        nc.vector.tensor_reduce(
                            out=M1[:, hh * hs:(hh + 1) * hs], in_=X[:, 0:hs, :],
                            axis=Ax.X, op=Alu.max)
                        Xf = X[:, 0:hs, :].rearrange("c h w -> c (h w)")
                        ce = hs * W // nchunks
                        for cchunk in range(nchunks):
                            nc.scalar.activation(
                                out=scr_act[:, 0:ce],
                                in_=Xf[:, cchunk * ce:(cchunk + 1) * ce],
                                func=Act.Copy,
                                accum_out=S_parts[:, hh * nchunks + cchunk:
                                                  hh * nchunks + cchunk + 1],
                            )
                    vmax = sp.tile([128, 1], f32, tag="vmax")
                    nc.vector.tensor_reduce(out=vmax[:], in_=M1[:], axis=Ax.X, op=Alu.max)
                    nc.vector.tensor_reduce(
                        out=S_b[:, ct:ct + 1], in_=S_parts[:], axis=Ax.X, op=Alu.add)
                    rowhit = sp.tile([128, H], f32, tag="rowhit")
                    nc.vector.tensor_scalar(
                        out=rowhit[:], in0=M1[:], scalar1=vmax[:],
                        scalar2=None, op0=Alu.is_ge,
                    )
                    nc.tensor.matmul(
                        psum_rc[:], cones[:], rowhit[:],
                        start=(ct == 0), stop=(ct == NT - 1),
                    )
                if inline_b0 and b < PB - 1:
                    epilogue(b)

            for b in range(PB):
                if not (inline_b0 and b < PB - 1):
                    epilogue(b)

    nc.compile()
    return nc


def _get_nc():
    global _NC
    if _NC is None:
        _NC = _build()
    return _NC


def _consts():
    thr8 = np.full((8, 1), -1.0, np.float32)
    for k in range(1, NS):
        thr8[k - 1, 0] = float((k * C) // NS)
    return {
        "cones": np.ones((128, 1), np.float32),
        "rones": np.ones((1, 128), np.float32),
        "ident": np.eye(128, dtype=np.float32),
        "thr8": thr8,
    }


def make_in_maps(x):
    c = _consts()
    return [dict(x=np.ascontiguousarray(x[PB * i:PB * (i + 1)]), **c) for i in range(NCORES)]


def kernel(**inputs):
    from concourse.bass_utils import run_bass_kernel_spmd

    x = np.asarray(inputs["x"], dtype=np.float32)
    assert x.shape == (B, C, H, W), x.shape
    nc = _get_nc()
    res = run_bass_kernel_spmd(nc, make_in_maps(x), core_ids=list(range(NCORES)))
    return np.concatenate([res.results[i]["out"] for i in range(NCORES)], axis=0)


# revision 35
# speedup vs baseline: 1.0570x; 1.0570x over previous
"""Trainium2 Bass kernel for the histogram_binning problem.

Input  x: [16, 512, 128, 128] f32.  Output: [16, 4096] f32.

Per batch b:
  vmax[c]  = max over (H,W) of x[b,c]
  rc[h]    = #channels whose argmax row is h (matches the reference's
             argmax-indicator row counts for tie-free-within-row data)
  Hc       = exclusive cumsum of rc
  bin edges hk[k] via the reference's sequential threshold scan, computed in
  closed form: J_k = #{t in 0..126 : HcInc[t] <= 64k}; a bin "fires" iff all
  previous fired, J strictly increases, and J_k <= 126.
  F[c]     = sum over (H,W) of x[b,c] / W
  out[b, k*512 + c] = F[c] / (hk[k+1] - hk[k])

Sharding: pure data parallel, 2 batches per core across 8 cores.

Per-core device pipeline (x shard [2, 512, 128, 128], streamed as 8 channel
tiles x 4 H-slices of [128c, 32h, 128w], 8-deep buffered, one gapless HWDGE
DMA stream on the sync ring):
  DMA in -> DVE row-max into M1 slices (concurrent) ACT copy-to-scratch with
  accum (per-channel sum S) -> DVE vmax -> DVE rowhit = (M1 >= vmax) -> PE
  matmul ones^T @ rowhit accumulating rc in PSUM.
Epilogues (both batches, deferred past the streaming loop so their latency
chain never starves the DMA queue): DVE scan for the cumulative histogram,
PE broadcast + one fused DVE compare-count for the 7 bin edges, fired-prefix
via mult-scan, reciprocal, PE broadcast of 1/hk_sub, PE transpose of the
[128c, 32] result block, one contiguous DMA out per batch.
"""

import numpy as np

B, C, H, W = 16, 512, 128, 128
NS = 8
NCORES = 8
PB = B // NCORES  # batches per core

_NC = None


def _build(xbufs=8, nh=4, inline_b0=False, scr_bf16=False, fold_last=False):
    from concourse import bacc, tile, mybir
    from concourse.bass_primitives import MemorySpace

    f32 = mybir.dt.float32
    Alu = mybir.AluOpType
    Act = mybir.ActivationFunctionType
    Ax = mybir.AxisListType

    nc = bacc.Bacc(None, target_bir_lowering=False)

    x_ext = nc.dram_tensor("x", [PB, C, H, W], f32, kind="ExternalInput")
    cones_ext = nc.dram_tensor("cones", [128, 1], f32, kind="ExternalInput")
    rones_ext = nc.dram_tensor("rones", [1, 128], f32, kind="ExternalInput")
    ident_ext = nc.dram_tensor("ident", [128, 128], f32, kind="ExternalInput")
    thr8_ext = nc.dram_tensor("thr8", [8, 1], f32, kind="ExternalInput")
    out_ext = nc.dram_tensor("out", [PB, NS * C], f32, kind="ExternalOutput")
    # out flat index = k*512 + q*128 + c with q = channel-tile index
    out_v = out_ext.rearrange("b (k q c) -> b q k c", k=NS, q=4, c=128)

    NT = C // 128  # channel tiles per batch

    with tile.TileContext(nc) as tc:
        with (
            tc.tile_pool(name="consts", bufs=1) as cp,
            tc.tile_pool(name="xp", bufs=xbufs) as xp,
            tc.tile_pool(name="sp", bufs=3) as sp,
            tc.tile_pool(name="ep", bufs=2) as ep,
            tc.tile_pool(name="pp", bufs=2, space=MemorySpace.PSUM) as pp,
        ):
            cones = cp.tile([128, 1], f32)
            nc.gpsimd.dma_start(out=cones[:], in_=cones_ext[:])
            rones = cp.tile([1, 128], f32)
            nc.gpsimd.dma_start(out=rones[:], in_=rones_ext[:])
            ident = cp.tile([128, 128], f32)
            nc.gpsimd.dma_start(out=ident[:], in_=ident_ext[:])
            thr8 = cp.tile([8, 1], f32)
            nc.gpsimd.dma_start(out=thr8[:], in_=thr8_ext[:])
            # ACT sum scratch: the Copy-with-accum needs a full-size out
            # stream; write it to a reusable scratch so X stays read-only
            # (keeps DVE and ACT concurrent on the same tile).
            NH = nh  # H-slices per channel tile (DMA granularity)
            HH = H // NH
            scr_dt = mybir.dt.bfloat16 if scr_bf16 else f32
            SCR_E = min(HH * W, 8192)
            scr_act = cp.tile([128, SCR_E], scr_dt)

            # ---- per-batch epilogues (deferred off the streaming path) ----
            def epilogue(b):
                S_b = S_bs[b]
                psum_rc = psum_rcs[b]
                HcInc = ep.tile([1, H], f32, tag="HcInc")
                nc.vector.tensor_tensor_scan(
                    out=HcInc[:], data0=psum_rc[:], data1=rones[:],
                    initial=0.0, op0=Alu.add, op1=Alu.mult,
                )
                # J_k = #{t in 0..H-2 : HcInc[t] <= 64k}: broadcast HcInc to 8
                # partitions (PE), one fused compare+count (DVE), transpose
                # back to the free axis (PE).
                psum_hc8 = pp.tile([8, H], f32, tag="hc8", bufs=1)
                nc.tensor.matmul(psum_hc8[:], rones[0:1, 0:8], HcInc[:],
                                 start=True, stop=True)
                J_pp = ep.tile([8, 1], f32, tag="J_pp")
                scr8 = ep.tile([8, H - 1], f32, tag="scr8")
                nc.vector.tensor_scalar(
                    out=scr8[:], in0=psum_hc8[0:8, 0:H - 1], scalar1=thr8[:],
                    scalar2=None, op0=Alu.is_le, op1=Alu.add,
                    accum_out=J_pp[:],
                )
                psum_J = pp.tile([1, 8], f32, tag="psum_J", bufs=1)
                nc.tensor.matmul(psum_J[:], J_pp[:], ident[0:8, 0:8],
                                 start=True, stop=True)
                Jb = ep.tile([1, 8], f32, tag="Jb")
                nc.vector.memset(Jb[:], 0.0)
                nc.vector.tensor_copy(out=Jb[0:1, 1:8], in_=psum_J[0:1, 0:7])
                inc = ep.tile([1, 7], f32, tag="inc")
                nc.vector.tensor_tensor(out=inc[:], in0=Jb[0:1, 1:8], in1=Jb[0:1, 0:7], op=Alu.is_gt)
                le = ep.tile([1, 7], f32, tag="le")
                nc.vector.tensor_scalar(
                    out=le[:], in0=Jb[0:1, 1:8], scalar1=float(H - 2),
                    scalar2=None, op0=Alu.is_le,
                )
                inc2 = ep.tile([1, 7], f32, tag="inc2")
                nc.vector.tensor_tensor(out=inc2[:], in0=inc[:], in1=le[:], op=Alu.mult)
                fired = ep.tile([1, 7], f32, tag="fired")
                nc.vector.tensor_tensor_scan(
                    out=fired[:], data0=inc2[:], data1=rones[0:1, 0:7],
                    initial=1.0, op0=Alu.mult, op1=Alu.mult,
                )
                hks = ep.tile([1, NS + 1], f32, tag="hks")
                nc.vector.memset(hks[:], 0.0)
                nc.vector.memset(hks[0:1, NS:NS + 1], float(H))
                nc.vector.tensor_tensor(out=hks[0:1, 1:8], in0=Jb[0:1, 1:8], in1=fired[:], op=Alu.mult)
                hsub = ep.tile([1, NS], f32, tag="hsub")
                nc.vector.tensor_tensor(out=hsub[:], in0=hks[0:1, 1:9], in1=hks[0:1, 0:8], op=Alu.subtract)
                inv = ep.tile([1, NS], f32, tag="inv")
                nc.vector.reciprocal(out=inv[:], in_=hsub[:])

                # broadcast inv across 128 partitions: [1,128] ones^T @ [1,8]
                psum_inv = pp.tile([128, NS], f32, tag="pinv", bufs=2)
                nc.tensor.matmul(psum_inv[:], rones[:], inv[:], start=True, stop=True)

                outt = ep.tile([128, NT, NS], f32, tag="outt")
                for ct in range(NT):
                    nc.vector.tensor_scalar(
                        out=outt[:, ct, :], in0=psum_inv[:],
                        scalar1=S_b[:, ct:ct + 1], scalar2=1.0 / float(W),
                        op0=Alu.mult, op1=Alu.mult,
                    )
                # transpose [128c, (q,k)=32] -> [32, 128c] via PE with identity
                psum_o = pp.tile([NT * NS, 128], f32, tag="po", bufs=2)
                nc.tensor.matmul(psum_o[:], outt[:], ident[:], start=True, stop=True)
                outT = ep.tile([NT * NS, 128], f32, tag="outT")
                nc.vector.tensor_copy(out=outT[:], in_=psum_o[:])
                nc.sync.dma_start(out=out_v[b], in_=outT[:])


            S_bs, psum_rcs = [], []
            for b in range(PB):
                S_b = ep.tile([128, NT], f32, tag="S_b")
                psum_rc = pp.tile([1, H], f32, tag="rc", bufs=2)
                S_bs.append(S_b)
                psum_rcs.append(psum_rc)
                for ct in range(NT):
                    # finer slices on the last tile -> shorter pipeline trail
                    nh = 8 if (b == PB - 1 and ct == NT - 1) else NH
                    hs = H // nh
                    M1 = sp.tile([128, H], f32, tag="M1")
                    nchunks = max(1, (H * W // nh) // SCR_E)
                    S_parts = sp.tile([128, 8], f32, tag="S_parts")
                    if nh * nchunks < 8:
                        nc.vector.memset(S_parts[:, nh * nchunks:8], 0.0)
                    assert nh * nchunks <= 8
                    lastt = (b == PB - 1 and ct == NT - 1)
                    for hh in range(nh):
                        X = xp.tile([128, HH, W], f32, tag="X")
                        nc.sync.dma_start(
                            out=X[:, 0:hs, :],
                            in_=x_ext[b, ct * 128:(ct + 1) * 128,
                                      hh * hs:(hh + 1) * hs])
                        if fold_last and lastt:
                            # W-fold on the otherwise-idle GpSimd so the
                            # final tile's row-max trail on DVE halves
                            XH = sp.tile([128, H // 8, W // 2], f32, tag="XH")
                            nc.gpsimd.tensor_tensor(
                                out=XH[:, 0:hs, :], in0=X[:, 0:hs, 0:W // 2],
                                in1=X[:, 0:hs, W // 2:W], op=Alu.max)
                            nc.vector.tensor_reduce(
                                out=M1[:, hh * hs:(hh + 1) * hs],
                                in_=XH[:, 0:hs, :], axis=Ax.X, op=Alu.max)
                        else:
                            nc.vector.tensor_reduce(
                                out=M1[:, hh * hs:(hh + 1) * hs], in_=X[:, 0:hs, :],
                                axis=Ax.X, op=Alu.max)
                        Xf = X[:, 0:hs, :].rearrange("c h w -> c (h w)")
                        ce = hs * W // nchunks
                        for cchunk in range(nchunks):
                            nc.scalar.activation(
                                out=scr_act[:, 0:ce],
                                in_=Xf[:, cchunk * ce:(cchunk + 1) * ce],
                                func=Act.Copy,
                                accum_out=S_parts[:, hh * nchunks + cchunk:
                                                  hh * nchunks + cchunk + 1],
                            )
                    vmax = sp.tile([128, 1], f32, tag="vmax")
                    nc.vector.tensor_reduce(out=vmax[:], in_=M1[:], axis=Ax.X, op=Alu.max)
                    nc.vector.tensor_reduce(
                        out=S_b[:, ct:ct + 1], in_=S_parts[:], axis=Ax.X, op=Alu.add)
                    rowhit = sp.tile([128, H], f32, tag="rowhit")
                    nc.vector.tensor_scalar(
                        out=rowhit[:], in0=M1[:], scalar1=vmax[:],
                        scalar2=None, op0=Alu.is_ge,
                    )
                    nc.tensor.matmul(
                        psum_rc[:], cones[:], rowhit[:],
                        start=(ct == 0), stop=(ct == NT - 1),
                    )
                if inline_b0 and b < PB - 1:
                    epilogue(b)

            for b in range(PB):
                if not (inline_b0 and b < PB - 1):
                    epilogue(b)

    nc.compile()
    return nc


def _get_nc():
    global _NC
    if _NC is None:
        _NC = _build()
    return _NC


def _consts():
    thr8 = np.full((8, 1), -1.0, np.float32)
    for k in range(1, NS):
        thr8[k - 1, 0] = float((k * C) // NS)
    return {
        "cones": np.ones((128, 1), np.float32),
        "rones": np.ones((1, 128), np.float32),
        "ident": np.eye(128, dtype=np.float32),
        "thr8": thr8,
    }


def make_in_maps(x):
    c = _consts()
    return [dict(x=np.ascontiguousarray(x[PB * i:PB * (i + 1)]), **c) for i in range(NCORES)]


def kernel(**inputs):
    from concourse.bass_utils import run_bass_kernel_spmd

    x = np.asarray(inputs["x"], dtype=np.float32)
    assert x.shape == (B, C, H, W), x.shape
    nc = _get_nc()
    res = run_bass_kernel_spmd(nc, make_in_maps(x), core_ids=list(range(NCORES)))
    return np.concatenate([res.results[i]["out"] for i in range(NCORES)], axis=0)
